# revision 1
# baseline (speedup 1.0000x reference)
"""MatchingNet model kernel for 8 Trainium2 NeuronCores.

Computation (reference semantics, N=4096, E=512, G=256, V=50000, R=1000):
  x  = embedding[input]          (N, E)
  ex = embedding[set_inputs]     (2, N, E)
  g_out = bidirectional 2-step LSTM over ex   (2, N, E)
  fh = lstm_f(x) + x             (N, E)          [single step, zero state]
  scores[b] = g_out[b] @ fh.T    (2, N, N)
  a = softmax(scores, axis=0)    -- softmax over b (size 2), pointwise in (n,m)
  r[b] = a[b] @ g_out[b]         (2, N, E)
  dot/nr/ng reductions over n -> cos (2, E) -> tiny tail -> softmax (R,)

Sharding: data-parallel over N. Core k owns rows [512k, 512k+512).
Each core runs the LSTMs for its own rows, then:
  AG1 all-gathers fh.T (keys), AG2 all-gathers g0/g1 (values).
  Attention trick: a[0] = sigmoid(D), a[1] = 1 - a[0] with
  D = (g0 - g1) @ fh.T, so only 3 N*N*E matmuls instead of 4.
Each core computes its n-shard of D and r, reduces over its n rows to
partials dot/sr/sg (2, E); the host sums partials and runs the O(R*E)
tail (cosine normalize, projection collapse, final softmax).
"""

import os
import sys

import numpy as np

for _p in ("/opt/trn_rl_repo", os.path.expanduser("~/.axon_site/_ro/trn_rl_repo")):
    if os.path.isdir(_p) and _p not in sys.path:
        sys.path.insert(0, _p)

import concourse.bacc as bacc
import concourse.bass as bass
import concourse.mybir as mybir
import concourse.tile as tile
from concourse import bass_utils
from concourse.masks import make_identity

N, E, G, V, R = 4096, 512, 256, 50000, 1000
NCORES = 8
NL = N // NCORES  # 512 rows per core
P = 128
NE = E // P   # 4 e-chunks
NH = G // P   # 2 hidden chunks for the g-LSTM
EPS = 1e-8

F32 = mybir.dt.float32
F32R = mybir.dt.float32r
I32 = mybir.dt.int32
AF = mybir.ActivationFunctionType
ALU = mybir.AluOpType


def _mm(ap):
    """Matmul operands are stored as float32r tiles already."""
    return ap


def _lstm_cell(nc, pools, H, xT, W_sb, U_sb, hprevT, cprevT, bias_sb, h_out, c_out):
    """Emit one LSTM cell, transposed layout (feature on partition, n free).

    gates.T[j, n] = sum_e W.T[e, j] x.T[e, n] (+ sum_h U.T[h, j] hprev.T[h, n]) + b[j]
    xT: (P, 4, NL); W_sb: (P, 4, 4H); U_sb: (P, H//P, 4H) or None.
    h_out/c_out: (P, H//P, NL). When cprevT is None the forget gate is skipped
    (sigmoid(f) * 0 contributes nothing) and c = sig(i)*tanh(g).
    """
    pg, gp, tp = pools["pg"], pools["gates"], pools["tmp"]
    hc = H // P
    nj = 4 * H // P
    gb = [gp.tile([P, hc, NL], F32, tag=f"gate{g}", bufs=1, name=f"gb{g}")
          for g in range(4)]
    if c_out is None:
        assert cprevT is None
        c_out = gb[1]  # forget-gate buffer is unused for zero-state cells
    for jc in range(nj):
        g = jc // hc
        if cprevT is None and g == 1:
            continue  # forget gate unused with zero initial state
        ps = pg.tile([P, NL], F32, tag="pg", bufs=4, name="ps_gate")
        js = slice(jc * P, (jc + 1) * P)
        for kt in range(NE):
            nc.tensor.matmul(
                ps[:], _mm(W_sb[:, kt, js]), _mm(xT[:, kt, :]),
                start=(kt == 0), stop=(U_sb is None and kt == NE - 1))
        if U_sb is not None:
            for kt in range(hc):
                nc.tensor.matmul(
                    ps[:], _mm(U_sb[:, kt, js]), _mm(hprevT[:, kt, :]),
                    start=False, stop=(kt == hc - 1))
        func = AF.Tanh if g == 2 else AF.Sigmoid
        nc.scalar.activation(
            out=gb[g][:, jc % hc, :], in_=ps[:], func=func,
            bias=bias_sb[:, jc:jc + 1], scale=1.0)
    for s in range(hc):
        i_, g_, o_ = gb[0][:, s, :], gb[2][:, s, :], gb[3][:, s, :]
        if cprevT is None:
            nc.vector.tensor_mul(c_out[:, s, :], i_, g_)
        else:
            f_ = gb[1][:, s, :]
            ig = tp.tile([P, NL], F32, tag="ig", bufs=2, name="ig")
            nc.vector.tensor_mul(ig[:], i_, g_)
            nc.vector.tensor_mul(c_out[:, s, :], f_, cprevT[:, s, :])
            nc.vector.tensor_add(c_out[:, s, :], c_out[:, s, :], ig[:])
        tc_ = tp.tile([P, NL], F32, tag="tanhc", bufs=2, name="tanhc")
        nc.scalar.activation(out=tc_[:], in_=c_out[:, s, :], func=AF.Tanh)
        nc.vector.tensor_mul(h_out[:, s, :], o_, tc_[:])


def _gather_T(nc, pools, emb, idx_dram, ident, dstT):
    """Gather NL embedding rows and transpose into dstT (P, NE, NL)."""
    ip, rp, pt, cp = pools["idx"], pools["raw"], pools["pt"], pools["tmp"]
    for t in range(NL // P):
        idx_t = ip.tile([P, 1], I32, tag="idx", bufs=4, name="idx_t")
        nc.sync.dma_start(out=idx_t[:], in_=idx_dram[t * P:(t + 1) * P, :])
        raw = rp.tile([P, E], F32, tag="raw", bufs=4, name="raw")
        nc.gpsimd.indirect_dma_start(
            out=raw[:], out_offset=None, in_=emb[:],
            in_offset=bass.IndirectOffsetOnAxis(ap=idx_t[:, :1], axis=0))
        for et in range(NE):
            ptile = pt.tile([P, P], F32, tag="pt", bufs=4, name="ptile")
            nc.tensor.transpose(
                out=ptile[:], in_=raw[:, et * P:(et + 1) * P], identity=ident[:])
            nc.vector.tensor_copy(
                out=dstT[:, et, t * P:(t + 1) * P], in_=ptile[:])


def build_program():
    nc = bacc.Bacc("TRN2", target_bir_lowering=False, debug=False,
                   enable_asserts=False, num_devices=NCORES)
    dram = lambda name, shape, dt=F32, kind="ExternalInput": \
        nc.dram_tensor(name, shape, dt, kind=kind).ap()

    emb = dram("emb", [V, E])
    idx_x = dram("idx_x", [NL, 1], I32)
    idx_e0 = dram("idx_e0", [NL, 1], I32)
    idx_e1 = dram("idx_e1", [NL, 1], I32)
    # weights pre-laid-out on host as lhsT tiles [p, kt, j]
    wgf = dram("wgf", [P, NE, 4 * G], F32R)
    wgr = dram("wgr", [P, NE, 4 * G], F32R)
    ugf = dram("ugf", [P, NH, 4 * G], F32R)
    ugr = dram("ugr", [P, NH, 4 * G], F32R)
    wf = dram("wf", [P, NE, 4 * E], F32R)
    bgf = dram("bgf", [P, 8])
    bgr = dram("bgr", [P, 8])
    bf = dram("bf", [P, 16])
    out_dot = dram("out_dot", [2, E], kind="ExternalOutput")
    out_sr = dram("out_sr", [2, E], kind="ExternalOutput")
    out_sg = dram("out_sg", [2, E], kind="ExternalOutput")

    with tile.TileContext(nc) as tc:
        _emit(tc, locals())
    nc.compile()
    return nc


PHASE = int(os.environ.get("KBENCH_PHASE", "4"))


def _dummy_outs(nc, pool, T):
    z = pool.tile([P, 12], F32, name="zeros")
    nc.vector.memset(z[:], 0.0)
    for nm in ("out_dot", "out_sr", "out_sg"):
        for b in range(2):
            for et in range(NE):
                nc.sync.dma_start(out=T[nm][b, et * P:(et + 1) * P],
                                  in_=z[:, b * 6 + et:b * 6 + et + 1])


def _emit(tc, T):
    nc = tc.nc
    rg = [list(range(NCORES))]
    BF16 = mybir.dt.bfloat16
    from contextlib import ExitStack
    ctx = ExitStack()
    with ctx:
        glob = ctx.enter_context(tc.tile_pool(name="glob", bufs=1))
        dramp = ctx.enter_context(tc.tile_pool(name="dramp", bufs=1, space="DRAM"))

        ident = glob.tile([P, P], F32)
        make_identity(nc, ident)
        identr = glob.tile([P, P], F32R)
        nc.vector.tensor_copy(out=identr[:], in_=ident[:])

        # collective bounce buffers: fh/logits side fp32r, value side bf16
        ag1_src = dramp.tile([E, NL], F32R)                      # fh.T local
        ag1_dst = dramp.tile([NCORES * E, NL], F32R, addr_space="Shared")
        # declared with wide rows (fewer rows, same bytes) — collective cost
        # scales with descriptor rows; access via the narrow row view below
        ag2_src_w = dramp.tile([NL, 2 * E], BF16)
        ag2_dst_w = dramp.tile([NCORES * NL, 2 * E], BF16, addr_space="Shared")
        ag2_src = ag2_src_w.rearrange("a (r b) -> (a r) b", r=2)  # (2NL, E)
        ag2_dst = ag2_dst_w.rearrange("a (r b) -> (a r) b", r=2)

        # long-lived local activations
        g0T = glob.tile([P, NE, NL], F32R)
        g1T = glob.tile([P, NE, NL], F32R)
        dgT = glob.tile([P, NE, NL], F32R)

        with tc.tile_pool(name="wpool", bufs=1) as wp, \
             tc.tile_pool(name="acts", bufs=1) as ap_, \
             tc.tile_pool(name="gates", bufs=1) as gp, \
             tc.tile_pool(name="tmp", bufs=1) as tp, \
             tc.tile_pool(name="idx", bufs=1) as ip, \
             tc.tile_pool(name="raw", bufs=1) as rp, \
             tc.tile_pool(name="pg", bufs=1, space="PSUM") as pgp, \
             tc.tile_pool(name="pt", bufs=1, space="PSUM") as ptp:
            pools = {"pg": pgp, "gates": gp, "tmp": tp, "idx": ip,
                     "raw": rp, "pt": ptp}

            # -- e gathers first: the g-LSTM is the long pole to AG2 --
            e0T = ap_.tile([P, NE, NL], F32R)
            e1T = ap_.tile([P, NE, NL], F32R)
            _gather_T(nc, pools, T["emb"], T["idx_e0"], ident, e0T)
            _gather_T(nc, pools, T["emb"], T["idx_e1"], ident, e1T)
            w_sb = {}
            for nm, kt in (("wgf", NE), ("wgr", NE), ("ugf", NH), ("ugr", NH)):
                w_sb[nm] = wp.tile([P, kt, 4 * G], F32R, name=nm + "_sb")
                nc.sync.dma_start(out=w_sb[nm][:], in_=T[nm][:])
            for nm in ("bgf", "bgr"):
                w_sb[nm] = wp.tile([P, 8], F32, name=nm + "_sb")
                nc.sync.dma_start(out=w_sb[nm][:], in_=T[nm][:])

            # two independent chains: (fwd0 -> fwd1) and (rev1 -> rev0)
            cfT = ap_.tile([P, NH, NL], F32, name="cfT")
            crT = ap_.tile([P, NH, NL], F32, name="crT")
            c2T = ap_.tile([P, NH, NL], F32, name="c2T")
            c3T = ap_.tile([P, NH, NL], F32, name="c3T")
            hf0 = g0T[:, 0:NH, :]
            hf1 = g1T[:, 0:NH, :]
            hr1 = g1T[:, NH:NE, :]
            hr0 = g0T[:, NH:NE, :]
            _lstm_cell(nc, pools, G, e0T, w_sb["wgf"], None, None, None,
                       w_sb["bgf"], hf0, cfT)
            _lstm_cell(nc, pools, G, e1T, w_sb["wgr"], None, None, None,
                       w_sb["bgr"], hr1, crT)
            # -- x gather, f weights, f-LSTM, AG1 --
            xT = ap_.tile([P, NE, NL], F32R)
            _gather_T(nc, pools, T["emb"], T["idx_x"], ident, xT)
            wf_sb = wp.tile([P, NE, 4 * E], F32R)
            nc.sync.dma_start(out=wf_sb[:], in_=T["wf"][:])
            bf_sb = wp.tile([P, 16], F32)
            nc.sync.dma_start(out=bf_sb[:], in_=T["bf"][:])

            fhT = ap_.tile([P, NE, NL], F32R)
            _lstm_cell(nc, pools, E, xT, wf_sb, None, None, None, bf_sb, fhT,
                       None)
            for et in range(NE):
                nc.vector.tensor_add(fhT[:, et, :], fhT[:, et, :], xT[:, et, :])
                nc.sync.dma_start(
                    out=ag1_src[et * P:(et + 1) * P, :], in_=fhT[:, et, :])
            nc.gpsimd.collective_compute(
                "AllGather", ALU.bypass, replica_groups=rg,
                ins=[ag1_src[:].opt()], outs=[ag1_dst[:].opt()])
            _lstm_cell(nc, pools, G, e1T, w_sb["wgf"], w_sb["ugf"], hf0, cfT,
                       w_sb["bgf"], hf1, c2T)
            _lstm_cell(nc, pools, G, e0T, w_sb["wgr"], w_sb["ugr"], hr1, crT,
                       w_sb["bgr"], hr0, c3T)


            for et in range(NE):
                nc.vector.tensor_sub(dgT[:, et, :], g0T[:, et, :], g1T[:, et, :])

        # -- transpose g0/g1 into ag2_src (bf16) and fire AG2 early --
        NMB = N // P  # 32 m-blocks
        attn = ctx.enter_context(tc.tile_pool(name="attn", bufs=1))
        A0T = attn.tile([P, NMB, NL], mybir.dt.bfloat16)
        with tc.tile_pool(name="tps", bufs=1) as tsp, \
             tc.tile_pool(name="fin", bufs=1) as fin, \
             tc.tile_pool(name="fhk", bufs=1) as fkp, \
             tc.tile_pool(name="ptg", bufs=1, space="PSUM") as ptgp, \
             tc.tile_pool(name="pd", bufs=1, space="PSUM") as pdp:
            with tc.high_priority():
                for srcT, row0 in ((g0T, 0), (g1T, NL)):
                    for nt in range(NL // P):
                        ptile = ptgp.tile([P, E], F32R, tag="ptg", bufs=3,
                                          name="ptg")
                        for et in range(NE):
                            nc.tensor.transpose(
                                out=ptile[:, et * P:(et + 1) * P],
                                in_=srcT[:, et, nt * P:(nt + 1) * P],
                                identity=identr[:])
                        stile = tsp.tile([P, E], BF16, tag="tps", bufs=3,
                                         name="stile")
                        nc.vector.tensor_copy(out=stile[:],
                                              in_=ptile[:].bitcast(F32))
                        nc.sync.dma_start(
                            out=ag2_src[row0 + nt * P:row0 + (nt + 1) * P, :],
                            in_=stile[:])
                nc.gpsimd.collective_compute(
                    "AllGather", ALU.bypass, replica_groups=rg,
                    ins=[ag2_src_w[:].opt()], outs=[ag2_dst_w[:].opt()])

            # -- early reductions: sg_b = sum_n g_b^2 (local) --
            for b, gT in ((0, g0T), (1, g1T)):
                for et in range(NE):
                    scr3 = fin.tile([P, NL], F32, tag="scr3", bufs=2,
                                    name="scr3")
                    asg = fin.tile([P, 1], F32, tag="asg", bufs=2, name="asg")
                    nc.scalar.activation(out=scr3[:],
                                         in_=gT[:, et, :].bitcast(F32),
                                         func=AF.Square, accum_out=asg[:])
                    nc.sync.dma_start(out=T["out_sg"][b, et * P:(et + 1) * P],
                                      in_=asg[:])

            # -- phase D1: D.T = fh.T-blocks x dgT; A0 = sigmoid(D) (bf16) --
            for k in range(NCORES):
                fhk = fkp.tile([P, NE, NL], F32R, tag="fhk", bufs=3, name="fhk")
                nc.sync.dma_start(
                    out=fhk[:],
                    in_=ag1_dst[k * E:(k + 1) * E, :].rearrange(
                        "(et p) n -> p et n", p=P))
                for c in range(NL // P):
                    mb = k * (NL // P) + c
                    pd = pdp.tile([P, NL], F32, tag="pd", bufs=3, name="pd")
                    cs = slice(c * P, (c + 1) * P)
                    for et in range(NE):
                        nc.tensor.matmul(
                            pd[:], fhk[:, et, cs], dgT[:, et, :],
                            start=(et == 0), stop=(et == NE - 1))
                    nc.scalar.activation(
                        out=A0T[:, mb, :], in_=pd[:], func=AF.Sigmoid)

        # ---- phase D2: r0.T / r1.T accumulate over all m-blocks in PSUM ----
        with tc.tile_pool(name="gb", bufs=1) as gbp, \
             tc.tile_pool(name="pr", bufs=1, space="PSUM") as prp, \
             tc.tile_pool(name="fin2", bufs=1) as fin:
            BF = mybir.dt.bfloat16
            r0p = [prp.tile([P, NL], F32, tag=f"r0_{et}", name=f"r0_{et}")
                   for et in range(NE)]
            r1p = [prp.tile([P, NL], F32, tag=f"r1_{et}", name=f"r1_{et}")
                   for et in range(NE)]
            for mb in range(NMB):
                k, c = divmod(mb, NL // P)
                base = k * 2 * NL
                g0b = gbp.tile([P, E], BF, tag="g0b", bufs=4, name="g0b")
                nc.sync.dma_start(
                    out=g0b[:], in_=ag2_dst[base + c * P:base + (c + 1) * P, :])
                g1b = gbp.tile([P, E], BF, tag="g1b", bufs=4, name="g1b")
                nc.sync.dma_start(
                    out=g1b[:],
                    in_=ag2_dst[base + NL + c * P:base + NL + (c + 1) * P, :])
                a1 = gbp.tile([P, NL], BF, tag="a1", bufs=4, name="a1")
                nc.vector.tensor_scalar(
                    out=a1[:], in0=A0T[:, mb, :], scalar1=-1.0, scalar2=1.0,
                    op0=ALU.mult, op1=ALU.add)
                for et in range(NE):
                    es = slice(et * P, (et + 1) * P)
                    nc.tensor.matmul(
                        r0p[et][:], g0b[:, es], A0T[:, mb, :],
                        start=(mb == 0), stop=(mb == NMB - 1))
                    nc.tensor.matmul(
                        r1p[et][:], g1b[:, es], a1[:],
                        start=(mb == 0), stop=(mb == NMB - 1))

            # ---- phase E: dot and r^2 reductions over local n ----
            for b, (rp_, gT) in enumerate(((r0p, g0T), (r1p, g1T))):
                for et in range(NE):
                    scr2 = fin.tile([P, NL], F32, tag="scr2", bufs=4,
                                    name="scr2")
                    asr = fin.tile([P, 1], F32, tag="asr", bufs=4, name="asr")
                    nc.scalar.activation(out=scr2[:], in_=rp_[et][:],
                                         func=AF.Square, accum_out=asr[:])
                    nc.sync.dma_start(out=T["out_sr"][b, et * P:(et + 1) * P],
                                      in_=asr[:])
                scr = fin.tile([P, NE, NL], F32, tag="scr", bufs=2, name="scr")
                adot = fin.tile([P, NE, 1], F32, tag="adot", bufs=2,
                                name="adot")
                for et in range(NE):
                    nc.vector.tensor_mul(scr[:, et, :], rp_[et][:],
                                         gT[:, et, :].bitcast(F32))
                nc.vector.reduce_sum(out=adot[:], in_=scr[:],
                                     axis=mybir.AxisListType.X)
                for et in range(NE):
                    nc.sync.dma_start(out=T["out_dot"][b, et * P:(et + 1) * P],
                                      in_=adot[:, et, :])



_PROGRAM = None


def _get_program():
    global _PROGRAM
    if _PROGRAM is None:
        _PROGRAM = build_program()
    return _PROGRAM


def _prep_w(w):
    """(4H, E_in) torch-layout weight -> lhsT tiles [p, kt, 4H]."""
    wt = np.asarray(w, np.float32).T  # (E_in, 4H)
    e_in, fourh = wt.shape
    return np.ascontiguousarray(wt.reshape(e_in // P, P, fourh).transpose(1, 0, 2))


def _prep_b(b1, b2):
    s = (np.asarray(b1, np.float32) + np.asarray(b2, np.float32))
    return np.ascontiguousarray(s.reshape(-1, P).T)


def run_device(inputs, trace=False):
    """Shard inputs, run the 8-core SPMD program, return (results, bass results)."""
    nc = _get_program()
    emb = np.ascontiguousarray(np.asarray(inputs["embedding"], np.float32))
    iq = np.asarray(inputs["input"]).astype(np.int32).reshape(N, 1)
    ie = np.asarray(inputs["set_inputs"]).astype(np.int32)
    shared = {
        "emb": emb,
        "wgf": _prep_w(inputs["wih_gf"]), "wgr": _prep_w(inputs["wih_gr"]),
        "ugf": _prep_w(inputs["whh_gf"]), "ugr": _prep_w(inputs["whh_gr"]),
        "wf": _prep_w(inputs["wih_f"]),
        "bgf": _prep_b(inputs["bih_gf"], inputs["bhh_gf"]),
        "bgr": _prep_b(inputs["bih_gr"], inputs["bhh_gr"]),
        "bf": _prep_b(inputs["bih_f"], inputs["bhh_f"]),
    }
    in_maps = []
    for k in range(NCORES):
        sl = slice(k * NL, (k + 1) * NL)
        m = dict(shared)
        m["idx_x"] = np.ascontiguousarray(iq[sl])
        m["idx_e0"] = np.ascontiguousarray(ie[0, sl].reshape(NL, 1))
        m["idx_e1"] = np.ascontiguousarray(ie[1, sl].reshape(NL, 1))
        in_maps.append(m)
    res = bass_utils.run_bass_kernel_spmd(
        nc, in_maps, core_ids=list(range(NCORES)), trace=trace)
    return res


def kernel(**inputs):
    res = run_device(inputs)
    return host_tail(res, inputs)


def host_tail(res, inputs):
    dot = np.zeros((2, E), np.float64)
    sr = np.zeros((2, E), np.float64)
    sg = np.zeros((2, E), np.float64)
    for r in res.results:
        dot += r["out_dot"]
        sr += r["out_sr"]
        sg += r["out_sg"]
    nr = np.maximum(np.sqrt(sr), EPS)
    ng = np.maximum(np.sqrt(sg), EPS)
    cos = dot / (nr * ng)                        # (2, E)
    kern = cos / np.exp(cos).sum()
    w_out = np.asarray(inputs["w_out"], np.float64)
    b_out = np.asarray(inputs["b_out"], np.float64)
    k2 = kern @ w_out.T + b_out                  # (2, R)
    s = k2.sum(axis=1)                           # (2,)
    labels = np.asarray(inputs["set_labels"], np.float64)
    o = s[0] * labels[0] + s[1] * labels[1]      # (R,)
    o = np.exp(o - o.max())
    o /= o.sum()
    return o.astype(np.float32)



# revision 21
# speedup vs baseline: 1.8873x; 1.8873x over previous
"""MatchingNet model kernel for 8 Trainium2 NeuronCores (v2, fp8 exchange).

Computation (reference semantics, N=4096, E=512, G=256, V=50000, R=1000):
  x  = embedding[input]          (N, E)
  ex = embedding[set_inputs]     (2, N, E)
  g_out = bidirectional 2-step LSTM over ex   (2, N, E)
  fh = lstm_f(x) + x             (N, E)          [single step, zero state]
  scores[b] = g_out[b] @ fh.T    (2, N, N)
  a = softmax(scores, axis=0)    -- softmax over b (size 2), pointwise in (n,m)
  r[b] = a[b] @ g_out[b]         (2, N, E)
  dot/nr/ng reductions over n -> cos (2, E) -> tiny tail -> softmax (R,)

Sharding: data-parallel over N. Core k owns rows [512k, 512k+512).
a[0] = sigmoid(D), a[1] = 1 - a[0] with D = (g0 - g1) @ fh.T.
r0 = A0 @ g0.  r1 = s1 - q with q = A0 @ g1 and s1[e] = sum_m g1[m,e];
the s1-dependent parts of dot1/sr1 are reconstructed on the host from
per-core partials (t1 = local colsum of g1, sum_q, sq_q, dot1q), so the
device never materializes a[1].

Exchange (all fp8 e4m3, partition-major blocks of 2KB rows):
  AG1: fh.T   (256KB/core -> 2MB)   fired right after the f-LSTM
  AG2a: g0    (256KB/core -> 2MB)   fired right after g0's transposes
  AG2b: g1    (256KB/core -> 2MB)
D1/D2 matmuls run in fp8 DoubleRow mode (2 k-subtiles per call).
"""

import os
import sys

import numpy as np

for _p in ("/opt/trn_rl_repo", os.path.expanduser("~/.axon_site/_ro/trn_rl_repo")):
    if os.path.isdir(_p) and _p not in sys.path:
        sys.path.insert(0, _p)

import concourse.bacc as bacc
import concourse.bass as bass
import concourse.mybir as mybir
import concourse.tile as tile
from concourse import bass_utils
from concourse.masks import make_identity

N, E, G, V, R = 4096, 512, 256, 50000, 1000
NCORES = 8
NL = N // NCORES  # 512 rows per core
P = 128
NE = E // P   # 4 e-chunks
NH = G // P   # 2 hidden chunks for the g-LSTM
NMB = N // P  # 32 global m-blocks
EPS = 1e-8

F32 = mybir.dt.float32
BF16 = mybir.dt.bfloat16
FP8 = mybir.dt.float8e4
I32 = mybir.dt.int32
AF = mybir.ActivationFunctionType
ALU = mybir.AluOpType
DR = mybir.MatmulPerfMode.DoubleRow


def _lstm_cell(nc, pools, H, xT, W_sb, U_sb, hprevT, cprevT, bias_sb, h_out,
               c_out, gates=(0, 1, 2, 3), packed=(0, 1, 2, 3)):
    """Emit one LSTM cell, transposed layout (feature on partition, n free).

    W_sb: (P, NE, len(packed)*H) packed in `packed` gate order; U_sb likewise
    or None.  bias_sb: (P, len(packed)*H//P).  h_out/c_out: (P, H//P, NL).
    Zero-state cells pass gates without 1 (forget) and cprevT=None.
    """
    pg, gp, tp = pools["pg"], pools["gates"], pools["tmp"]
    hc = H // P
    pos = {g: i for i, g in enumerate(packed)}
    gb = {}
    for g in gates:
        gb[g] = gp.tile([P, hc, NL], F32, tag=f"gate{g}", bufs=1, name=f"gb{g}")
        for s in range(hc):
            jc = pos[g] * hc + s
            ps = pg.tile([P, NL], F32, tag="pg", bufs=4, name="ps_gate")
            js = slice(jc * P, (jc + 1) * P)
            for kt in range(NE):
                nc.tensor.matmul(
                    ps[:], W_sb[:, kt, js], xT[:, kt, :],
                    start=(kt == 0), stop=(U_sb is None and kt == NE - 1))
            if U_sb is not None:
                hcu = hprevT.shape[1]
                for kt in range(hcu):
                    nc.tensor.matmul(
                        ps[:], U_sb[:, kt, js], hprevT[:, kt, :],
                        start=False, stop=(kt == hcu - 1))
            func = AF.Tanh if g == 2 else AF.Sigmoid
            nc.scalar.activation(
                out=gb[g][:, s, :], in_=ps[:], func=func,
                bias=bias_sb[:, jc:jc + 1], scale=1.0)
    for s in range(hc):
        i_, g_, o_ = gb[0][:, s, :], gb[2][:, s, :], gb[3][:, s, :]
        if c_out is None:
            c_s = tp.tile([P, NL], F32, tag="ctmp", bufs=2, name="ctmp")
        else:
            c_s = c_out[:, s, :]
        if cprevT is None:
            nc.vector.tensor_mul(c_s, i_, g_)
        else:
            f_ = gb[1][:, s, :]
            ig = tp.tile([P, NL], F32, tag="ig", bufs=2, name="ig")
            nc.vector.tensor_mul(ig[:], i_, g_)
            nc.vector.tensor_mul(c_s, f_, cprevT[:, s, :])
            nc.vector.tensor_add(c_s, c_s, ig[:])
        tc_ = tp.tile([P, NL], F32, tag="tanhc", bufs=2, name="tanhc")
        nc.scalar.activation(out=tc_[:], in_=c_s, func=AF.Tanh)
        nc.vector.tensor_mul(h_out[:, s, :], o_, tc_[:])


def _gather_T(nc, pools, emb, idx_dram, ident, dstT):
    """Gather NL embedding rows and transpose into dstT (P, NE, NL)."""
    ip, rp, pt, cp = pools["idx"], pools["raw"], pools["pt"], pools["tmp"]
    for t in range(NL // P):
        idx_t = ip.tile([P, 1], I32, tag="idx", bufs=4, name="idx_t")
        nc.sync.dma_start(out=idx_t[:], in_=idx_dram[t * P:(t + 1) * P, :])
        raw = rp.tile([P, E], F32, tag="raw", bufs=4, name="raw")
        nc.gpsimd.indirect_dma_start(
            out=raw[:], out_offset=None, in_=emb[:],
            in_offset=bass.IndirectOffsetOnAxis(ap=idx_t[:, :1], axis=0))
        for et in range(NE):
            ptile = pt.tile([P, P], F32, tag="pt", bufs=4, name="ptile")
            nc.tensor.transpose(
                out=ptile[:], in_=raw[:, et * P:(et + 1) * P], identity=ident[:])
            nc.vector.tensor_copy(
                out=dstT[:, et, t * P:(t + 1) * P], in_=ptile[:])


def build_program():
    nc = bacc.Bacc("TRN2", target_bir_lowering=False, debug=False,
                   enable_asserts=False, num_devices=NCORES)
    dram = lambda name, shape, dt=F32, kind="ExternalInput": \
        nc.dram_tensor(name, shape, dt, kind=kind).ap()

    emb = dram("emb", [V, E])
    idx_x = dram("idx_x", [NL, 1], I32)
    idx_e0 = dram("idx_e0", [NL, 1], I32)
    idx_e1 = dram("idx_e1", [NL, 1], I32)
    # weights pre-laid-out on host as lhsT tiles [p, kt, j]
    wgf = dram("wgf", [P, NE, 4 * G], BF16)
    wgr = dram("wgr", [P, NE, 4 * G], BF16)
    ugf = dram("ugf", [P, NH, 4 * G], BF16)
    ugr = dram("ugr", [P, NH, 4 * G], BF16)
    wf = dram("wf", [P, NE, 3 * E], BF16)   # i, g, o gates only (zero state)
    bgf = dram("bgf", [P, 8])
    bgr = dram("bgr", [P, 8])
    bf = dram("bf", [P, 12])
    out_a = dram("out_a", [P, 24], kind="ExternalOutput")
    out_d = dram("out_d", [P, 8], kind="ExternalOutput")

    with tile.TileContext(nc) as tc:
        _emit(tc, locals())
    nc.compile()
    return nc


def _emit(tc, T):
    nc = tc.nc
    rg = [list(range(NCORES))]
    from contextlib import ExitStack
    ctx = ExitStack()
    with ctx:
        glob = ctx.enter_context(tc.tile_pool(name="glob", bufs=1))
        dramp = ctx.enter_context(tc.tile_pool(name="dramp", bufs=1, space="DRAM"))

        ident = glob.tile([P, P], F32)
        make_identity(nc, ident)
        identb = glob.tile([P, P], BF16)
        nc.vector.tensor_copy(out=identb[:], in_=ident[:])

        # fp8 exchange buffers, partition-major 2KB rows
        ag1_src = dramp.tile([P, NE * NL], FP8)
        ag1_dst = dramp.tile([NCORES * P, NE * NL], FP8, addr_space="Shared")
        ag2a_src = dramp.tile([P, 4 * E], FP8)
        ag2a_dst = dramp.tile([NCORES * P, 4 * E], FP8, addr_space="Shared")
        ag2b_src = dramp.tile([P, 4 * E], FP8)
        ag2b_dst = dramp.tile([NCORES * P, 4 * E], FP8, addr_space="Shared")

        # long-lived activations
        g0T = glob.tile([P, NE, NL], BF16)
        g1T = glob.tile([P, NE, NL], BF16)
        dg8 = glob.tile([P, NE, NL], FP8)
        A0T = glob.tile([P, NMB, NL], FP8)
        out_act = glob.tile([P, 24], F32)
        out_dve = glob.tile([P, 8], F32)

        with tc.tile_pool(name="wpool", bufs=1) as wp, \
             tc.tile_pool(name="acts", bufs=1) as ap_, \
             tc.tile_pool(name="gates", bufs=1) as gp, \
             tc.tile_pool(name="tmp", bufs=1) as tp, \
             tc.tile_pool(name="idx", bufs=1) as ip, \
             tc.tile_pool(name="raw", bufs=1) as rp, \
             tc.tile_pool(name="fhk", bufs=1) as fkp, \
             tc.tile_pool(name="tps", bufs=1) as tsp:
            lstm_psum = tc.tile_pool(name="pg", bufs=1, space="PSUM")
            pgp = lstm_psum.__enter__()
            ptp_cm = tc.tile_pool(name="pt", bufs=1, space="PSUM")
            ptp = ptp_cm.__enter__()
            pools = {"pg": pgp, "gates": gp, "tmp": tp, "idx": ip,
                     "raw": rp, "pt": ptp}

            # ---- phase F: x gather, f-LSTM, AG1 as early as possible ----
            xT = ap_.tile([P, NE, NL], BF16)
            _gather_T(nc, pools, T["emb"], T["idx_x"], ident, xT)
            e0T = ap_.tile([P, NE, NL], BF16)
            e1T = ap_.tile([P, NE, NL], BF16)
            _gather_T(nc, pools, T["emb"], T["idx_e0"], ident, e0T)
            _gather_T(nc, pools, T["emb"], T["idx_e1"], ident, e1T)

            wf_sb = wp.tile([P, NE, 3 * E], BF16)
            for q in range(3):   # chunked so the i-gate matmuls start early
                qs = slice(q * E, (q + 1) * E)
                nc.sync.dma_start(out=wf_sb[:, :, qs], in_=T["wf"][:, :, qs])
            bf_sb = wp.tile([P, 12], F32)
            nc.sync.dma_start(out=bf_sb[:], in_=T["bf"][:])
            w_sb = {}
            for nm, kt in (("wgf", NE), ("wgr", NE), ("ugf", NH), ("ugr", NH)):
                w_sb[nm] = wp.tile([P, kt, 4 * G], BF16, name=nm + "_sb")
                for q in (0, 2, 3, 1):   # forget gate last (needed by stage 2)
                    qs = slice(q * G, (q + 1) * G)
                    nc.sync.dma_start(out=w_sb[nm][:, :, qs],
                                      in_=T[nm][:, :, qs])
            for nm in ("bgf", "bgr"):
                w_sb[nm] = wp.tile([P, 8], F32, name=nm + "_sb")
                nc.sync.dma_start(out=w_sb[nm][:], in_=T[nm][:])

            fhT = ap_.tile([P, NE, NL], BF16)
            _lstm_cell(nc, pools, E, xT, wf_sb, None, None, None, bf_sb, fhT,
                       None, gates=(0, 2, 3), packed=(0, 2, 3))
            fh8 = ap_.tile([P, NE, NL], FP8, name="fh8")
            for et in range(NE):
                nc.vector.tensor_add(fhT[:, et, :], fhT[:, et, :], xT[:, et, :])
                nc.vector.tensor_copy(out=fh8[:, et, :], in_=fhT[:, et, :])
            nc.sync.dma_start(
                out=ag1_src[:], in_=fh8[:].rearrange("p et n -> p (et n)"))
            nc.gpsimd.collective_compute(
                "AllGather", ALU.bypass, replica_groups=rg,
                ins=[ag1_src[:].opt()], outs=[ag1_dst[:].opt()])

            # ---- g-LSTM: stage 1 (zero state), then stage 2 ----
            cfT = ap_.tile([P, NH, NL], F32, name="cfT")
            crT = ap_.tile([P, NH, NL], F32, name="crT")
            c2T = ap_.tile([P, NH, NL], F32, name="c2T")
            c3T = ap_.tile([P, NH, NL], F32, name="c3T")
            hf0 = g0T[:, 0:NH, :]
            hf1 = g1T[:, 0:NH, :]
            hr1 = g1T[:, NH:NE, :]
            hr0 = g0T[:, NH:NE, :]
            _lstm_cell(nc, pools, G, e0T, w_sb["wgf"], None, None, None,
                       w_sb["bgf"], hf0, cfT, gates=(0, 2, 3))
            _lstm_cell(nc, pools, G, e1T, w_sb["wgr"], None, None, None,
                       w_sb["bgr"], hr1, crT, gates=(0, 2, 3))
            # forget-gate weight quarters land last; stage 2 needs them
            _lstm_cell(nc, pools, G, e1T, w_sb["wgf"], w_sb["ugf"], hf0, cfT,
                       w_sb["bgf"], hf1, c2T)
            # dg e-subtiles 0,1 (hf0-hf1) ready before rev0 completes
            for et in range(NH):
                nc.vector.tensor_sub(dg8[:, et, :], g0T[:, et, :],
                                     g1T[:, et, :])
            _lstm_cell(nc, pools, G, e0T, w_sb["wgr"], w_sb["ugr"], hr1, crT,
                       w_sb["bgr"], hr0, c3T)
            for et in range(NH, NE):
                nc.vector.tensor_sub(dg8[:, et, :], g0T[:, et, :],
                                     g1T[:, et, :])
            ptp_cm.__exit__(None, None, None)
            lstm_psum.__exit__(None, None, None)
            pd_cm = tc.tile_pool(name="pd", bufs=1, space="PSUM")
            pdp = pd_cm.__enter__()

            # ---- transpose g0/g1 to row-major fp8, fire AG2a / AG2b ----
            with tc.high_priority():
                for srcT, s_sb, a_src, a_dst in (
                        (g0T, "s0", ag2a_src, ag2a_dst),
                        (g1T, "s1", ag2b_src, ag2b_dst)):
                    s_t = tsp.tile([P, NE, E], FP8, tag=s_sb, bufs=1, name=s_sb)
                    for nt in range(NL // P):
                        ptile = pdp.tile([P, E], BF16, tag="ptg", bufs=3,
                                         name="ptg")
                        for et in range(NE):
                            nc.tensor.transpose(
                                out=ptile[:, et * P:(et + 1) * P],
                                in_=srcT[:, et, nt * P:(nt + 1) * P],
                                identity=identb[:])
                        nc.vector.tensor_copy(out=s_t[:, nt, :], in_=ptile[:])
                    nc.sync.dma_start(
                        out=a_src[:], in_=s_t[:].rearrange("p s e -> p (s e)"))
                    nc.gpsimd.collective_compute(
                        "AllGather", ALU.bypass, replica_groups=rg,
                        ins=[a_src[:].opt()], outs=[a_dst[:].opt()])

            # ---- D1: A0 = sigmoid((g0-g1) @ fh_all.T), fp8 DoubleRow ----
            for k in range(NCORES):
                fhk = fkp.tile([P, NE, NL], FP8, tag="fhk", bufs=3, name="fhk")
                nc.sync.dma_start(
                    out=fhk[:],
                    in_=ag1_dst[k * P:(k + 1) * P, :].rearrange(
                        "p (et n) -> p et n", et=NE))
                for c in range(NL // P):
                    mb = k * (NL // P) + c
                    pd = pdp.tile([P, NL], F32, tag="pd", bufs=3, name="pd")
                    cs = slice(c * P, (c + 1) * P)
                    nc.tensor.matmul(pd[:], fhk[:, 0:2, cs], dg8[:, 0:2, :],
                                     start=True, stop=False, perf_mode=DR)
                    nc.tensor.matmul(pd[:], fhk[:, 2:4, cs], dg8[:, 2:4, :],
                                     start=False, stop=True, perf_mode=DR)
                    nc.scalar.activation(
                        out=A0T[:, mb, :], in_=pd[:], func=AF.Sigmoid)
            pd_cm.__exit__(None, None, None)

        # ---- D2: r0 = A0@g0, q = A0@g1 (fp8 DoubleRow), reductions ----
        with tc.tile_pool(name="gb", bufs=1) as gbp, \
             tc.tile_pool(name="fin", bufs=1) as fin, \
             tc.tile_pool(name="pr", bufs=1, space="PSUM") as prp:
            for phase, (a_dst, gT, dcol0) in enumerate(
                    ((ag2a_dst, g0T, 0), (ag2b_dst, g1T, 4))):
                rp_ = [prp.tile([P, NL], F32, tag=f"r{phase}_{et}",
                                name=f"r{phase}_{et}") for et in range(NE)]
                for k in range(NCORES):
                    gpk = gbp.tile([P, NE, E], FP8, tag="gpk", bufs=4,
                                   name="gpk")
                    nc.sync.dma_start(
                        out=gpk[:],
                        in_=a_dst[k * P:(k + 1) * P, :].rearrange(
                            "p (s e) -> p s e", s=NE))
                    for cp in range(2):
                        mp = k * 4 + 2 * cp
                        for et in range(NE):
                            nc.tensor.matmul(
                                rp_[et][:],
                                gpk[:, 2 * cp:2 * cp + 2, et * P:(et + 1) * P],
                                A0T[:, mp:mp + 2, :],
                                start=(k == 0 and cp == 0),
                                stop=(k == NCORES - 1 and cp == 1),
                                perf_mode=DR)
                # per-et reductions: sr (Act), dot (DVE), and for q: sum_q
                for et in range(NE):
                    scr = fin.tile([P, NL], F32, tag="scr", bufs=4, name="scr")
                    nc.scalar.activation(
                        out=scr[:], in_=rp_[et][:], func=AF.Square,
                        accum_out=out_act[:, 8 * phase + et:8 * phase + et + 1])
                    scr2 = fin.tile([P, NL], F32, tag="scr2", bufs=4,
                                    name="scr2")
                    nc.vector.scalar_tensor_tensor(
                        out=scr2[:], in0=rp_[et][:], scalar=1.0,
                        in1=gT[:, et, :],
                        op0=ALU.mult, op1=ALU.mult,
                        accum_out=out_dve[:, dcol0 + et:dcol0 + et + 1])
                    if phase == 1:
                        scr3 = fin.tile([P, NL], F32, tag="scr3", bufs=4,
                                        name="scr3")
                        nc.scalar.activation(
                            out=scr3[:], in_=rp_[et][:], func=AF.Copy,
                            accum_out=out_act[:, 4 + et:4 + et + 1])

            # sg and t1 reductions from local g (fp32)
            for b, gT in ((0, g0T), (1, g1T)):
                for et in range(NE):
                    scr = fin.tile([P, NL], F32, tag="scr", bufs=4, name="scr")
                    c0 = 12 + 4 * b + et
                    nc.scalar.activation(
                        out=scr[:], in_=gT[:, et, :],
                        func=AF.Square, accum_out=out_act[:, c0:c0 + 1])
            for et in range(NE):
                scr = fin.tile([P, NL], F32, tag="scr", bufs=4, name="scr")
                nc.scalar.activation(
                    out=scr[:], in_=g1T[:, et, :],
                    func=AF.Copy, accum_out=out_act[:, 20 + et:20 + et + 1])

            nc.sync.dma_start(out=T["out_a"][:], in_=out_act[:])
            nc.sync.dma_start(out=T["out_d"][:], in_=out_dve[:])


_PROGRAM = None


def _get_program():
    global _PROGRAM
    if _PROGRAM is None:
        _PROGRAM = build_program()
    return _PROGRAM


def _prep_w(w, gates=(0, 1, 2, 3)):
    """(4H, E_in) torch-layout weight -> bf16 lhsT tiles [p, kt, len(gates)*H]."""
    import ml_dtypes
    w = np.asarray(w, np.float32)
    h4 = w.shape[0]
    h = h4 // 4
    wt = np.concatenate([w[g * h:(g + 1) * h] for g in gates], 0).T
    e_in, cols = wt.shape
    return np.ascontiguousarray(
        wt.reshape(e_in // P, P, cols).transpose(1, 0, 2)
        .astype(ml_dtypes.bfloat16))


def _prep_b(b1, b2, gates=(0, 1, 2, 3)):
    s = np.asarray(b1, np.float32) + np.asarray(b2, np.float32)
    h = s.shape[0] // 4
    s = np.concatenate([s[g * h:(g + 1) * h] for g in gates], 0)
    return np.ascontiguousarray(s.reshape(-1, P).T)


def run_device(inputs, trace=False):
    """Shard inputs, run the 8-core SPMD program, return bass results."""
    nc = _get_program()
    emb = np.ascontiguousarray(np.asarray(inputs["embedding"], np.float32))
    iq = np.asarray(inputs["input"]).astype(np.int32).reshape(N, 1)
    ie = np.asarray(inputs["set_inputs"]).astype(np.int32)
    shared = {
        "emb": emb,
        "wgf": _prep_w(inputs["wih_gf"]), "wgr": _prep_w(inputs["wih_gr"]),
        "ugf": _prep_w(inputs["whh_gf"]), "ugr": _prep_w(inputs["whh_gr"]),
        "wf": _prep_w(inputs["wih_f"], gates=(0, 2, 3)),
        "bgf": _prep_b(inputs["bih_gf"], inputs["bhh_gf"]),
        "bgr": _prep_b(inputs["bih_gr"], inputs["bhh_gr"]),
        "bf": _prep_b(inputs["bih_f"], inputs["bhh_f"], gates=(0, 2, 3)),
    }
    in_maps = []
    for k in range(NCORES):
        sl = slice(k * NL, (k + 1) * NL)
        m = dict(shared)
        m["idx_x"] = np.ascontiguousarray(iq[sl])
        m["idx_e0"] = np.ascontiguousarray(ie[0, sl].reshape(NL, 1))
        m["idx_e1"] = np.ascontiguousarray(ie[1, sl].reshape(NL, 1))
        in_maps.append(m)
    res = bass_utils.run_bass_kernel_spmd(
        nc, in_maps, core_ids=list(range(NCORES)), trace=trace)
    return res


def kernel(**inputs):
    res = run_device(inputs)
    return host_tail(res, inputs)


def host_tail(res, inputs):
    # device partial layout:
    #   out_a cols: sr0(0:4) sqq(4:8)... see below; out_d cols: dot0, dot1q
    # column c of kind at base: value for e = et*128 + p
    sr0 = np.zeros(E, np.float64)
    sqq = np.zeros(E, np.float64)
    sumq = np.zeros(E, np.float64)
    sg0 = np.zeros(E, np.float64)
    sg1 = np.zeros(E, np.float64)
    t1 = np.zeros(E, np.float64)
    dot0 = np.zeros(E, np.float64)
    dot1q = np.zeros(E, np.float64)
    for r in res.results:
        a = np.asarray(r["out_a"], np.float64)   # (P, 24)
        d = np.asarray(r["out_d"], np.float64)   # (P, 8)
        for et in range(NE):
            sl = slice(et * P, (et + 1) * P)
            sr0[sl] += a[:, 0 + et]
            sumq[sl] += a[:, 4 + et]
            sqq[sl] += a[:, 8 + et]
            sg0[sl] += a[:, 12 + et]
            sg1[sl] += a[:, 16 + et]
            t1[sl] += a[:, 20 + et]
            dot0[sl] += d[:, et]
            dot1q[sl] += d[:, 4 + et]
    s1 = t1
    dot1 = s1 * t1 - dot1q
    sr1 = N * s1 ** 2 - 2.0 * s1 * sumq + sqq
    dot = np.stack([dot0, dot1])
    sr = np.stack([sr0, sr1])
    sg = np.stack([sg0, sg1])
    nr = np.maximum(np.sqrt(sr), EPS)
    ng = np.maximum(np.sqrt(sg), EPS)
    cos = dot / (nr * ng)
    kern = cos / np.exp(cos).sum()
    w_out = np.asarray(inputs["w_out"], np.float64)
    b_out = np.asarray(inputs["b_out"], np.float64)
    k2 = kern @ w_out.T + b_out
    s = k2.sum(axis=1)
    labels = np.asarray(inputs["set_labels"], np.float64)
    o = s[0] * labels[0] + s[1] * labels[1]
    o = np.exp(o - o.max())
    o /= o.sum()
    return o.astype(np.float32)


# revision 24
# speedup vs baseline: 2.0738x; 1.0989x over previous
"""MatchingNet model kernel for 8 Trainium2 NeuronCores (v2, fp8 exchange).

Computation (reference semantics, N=4096, E=512, G=256, V=50000, R=1000):
  x  = embedding[input]          (N, E)
  ex = embedding[set_inputs]     (2, N, E)
  g_out = bidirectional 2-step LSTM over ex   (2, N, E)
  fh = lstm_f(x) + x             (N, E)          [single step, zero state]
  scores[b] = g_out[b] @ fh.T    (2, N, N)
  a = softmax(scores, axis=0)    -- softmax over b (size 2), pointwise in (n,m)
  r[b] = a[b] @ g_out[b]         (2, N, E)
  dot/nr/ng reductions over n -> cos (2, E) -> tiny tail -> softmax (R,)

Sharding: data-parallel over N. Core k owns rows [512k, 512k+512).
a[0] = sigmoid(D), a[1] = 1 - a[0] with D = (g0 - g1) @ fh.T.
r0 = A0 @ g0.  r1 = s1 - q with q = A0 @ g1 and s1[e] = sum_m g1[m,e];
the s1-dependent parts of dot1/sr1 are reconstructed on the host from
per-core partials (t1 = local colsum of g1, sum_q, sq_q, dot1q), so the
device never materializes a[1].

Exchange (all fp8 e4m3, partition-major blocks of 2KB rows):
  AG1: fh.T   (256KB/core -> 2MB)   fired right after the f-LSTM
  AG2a: g0    (256KB/core -> 2MB)   fired right after g0's transposes
  AG2b: g1    (256KB/core -> 2MB)
D1/D2 matmuls run in fp8 DoubleRow mode (2 k-subtiles per call).
"""

import os
import sys

import numpy as np

for _p in ("/opt/trn_rl_repo", os.path.expanduser("~/.axon_site/_ro/trn_rl_repo")):
    if os.path.isdir(_p) and _p not in sys.path:
        sys.path.insert(0, _p)

import concourse.bacc as bacc
import concourse.bass as bass
import concourse.mybir as mybir
import concourse.tile as tile
from concourse import bass_utils
from concourse.masks import make_identity

N, E, G, V, R = 4096, 512, 256, 50000, 1000
NCORES = 8
NL = N // NCORES  # 512 rows per core
P = 128
NE = E // P   # 4 e-chunks
NH = G // P   # 2 hidden chunks for the g-LSTM
NMB = N // P  # 32 global m-blocks
EPS = 1e-8

F32 = mybir.dt.float32
BF16 = mybir.dt.bfloat16
FP8 = mybir.dt.float8e4
I32 = mybir.dt.int32
AF = mybir.ActivationFunctionType
ALU = mybir.AluOpType
DR = mybir.MatmulPerfMode.DoubleRow


def _lstm_cell(nc, pools, H, xT, W_sb, U_sb, hprevT, cprevT, bias_sb, h_out,
               c_out, gates=(0, 1, 2, 3), packed=(0, 1, 2, 3)):
    """Emit one LSTM cell, transposed layout (feature on partition, n free).

    W_sb: (P, NE, len(packed)*H) packed in `packed` gate order; U_sb likewise
    or None.  bias_sb: (P, len(packed)*H//P).  h_out/c_out: (P, H//P, NL).
    Zero-state cells pass gates without 1 (forget) and cprevT=None.
    """
    pg, gp, tp = pools["pg"], pools["gates"], pools["tmp"]
    hc = H // P
    pos = {g: i for i, g in enumerate(packed)}
    gb = {}
    for g in gates:
        gb[g] = gp.tile([P, hc, NL], F32, tag=f"gate{g}", bufs=1, name=f"gb{g}")
        for s in range(hc):
            jc = pos[g] * hc + s
            ps = pg.tile([P, NL], F32, tag="pg", bufs=4, name="ps_gate")
            js = slice(jc * P, (jc + 1) * P)
            for kt in range(NE):
                nc.tensor.matmul(
                    ps[:], W_sb[:, kt, js], xT[:, kt, :],
                    start=(kt == 0), stop=(U_sb is None and kt == NE - 1))
            if U_sb is not None:
                hcu = hprevT.shape[1]
                for kt in range(hcu):
                    nc.tensor.matmul(
                        ps[:], U_sb[:, kt, js], hprevT[:, kt, :],
                        start=False, stop=(kt == hcu - 1))
            func = AF.Tanh if g == 2 else AF.Sigmoid
            nc.scalar.activation(
                out=gb[g][:, s, :], in_=ps[:], func=func,
                bias=bias_sb[:, jc:jc + 1], scale=1.0)
    for s in range(hc):
        i_, g_, o_ = gb[0][:, s, :], gb[2][:, s, :], gb[3][:, s, :]
        if c_out is None:
            c_s = tp.tile([P, NL], F32, tag="ctmp", bufs=2, name="ctmp")
        else:
            c_s = c_out[:, s, :]
        if cprevT is None:
            nc.vector.tensor_mul(c_s, i_, g_)
        else:
            f_ = gb[1][:, s, :]
            ig = tp.tile([P, NL], F32, tag="ig", bufs=2, name="ig")
            nc.vector.tensor_mul(ig[:], i_, g_)
            nc.vector.tensor_mul(c_s, f_, cprevT[:, s, :])
            nc.vector.tensor_add(c_s, c_s, ig[:])
        tc_ = tp.tile([P, NL], F32, tag="tanhc", bufs=2, name="tanhc")
        nc.scalar.activation(out=tc_[:], in_=c_s, func=AF.Tanh)
        nc.vector.tensor_mul(h_out[:, s, :], o_, tc_[:])


def _gather_T(nc, pools, emb, idx_dram, ident, dstT):
    """Gather NL embedding rows and transpose into dstT (P, NE, NL)."""
    ip, rp, pt, cp = pools["idx"], pools["raw"], pools["pt"], pools["tmp"]
    for t in range(NL // P):
        idx_t = ip.tile([P, 1], I32, tag="idx", bufs=4, name="idx_t")
        nc.sync.dma_start(out=idx_t[:], in_=idx_dram[t * P:(t + 1) * P, :])
        raw = rp.tile([P, E], F32, tag="raw", bufs=4, name="raw")
        nc.gpsimd.indirect_dma_start(
            out=raw[:], out_offset=None, in_=emb[:],
            in_offset=bass.IndirectOffsetOnAxis(ap=idx_t[:, :1], axis=0))
        for et in range(NE):
            ptile = pt.tile([P, P], F32, tag="pt", bufs=4, name="ptile")
            nc.tensor.transpose(
                out=ptile[:], in_=raw[:, et * P:(et + 1) * P], identity=ident[:])
            nc.vector.tensor_copy(
                out=dstT[:, et, t * P:(t + 1) * P], in_=ptile[:])


def build_program():
    nc = bacc.Bacc("TRN2", target_bir_lowering=False, debug=False,
                   enable_asserts=False, num_devices=NCORES)
    dram = lambda name, shape, dt=F32, kind="ExternalInput": \
        nc.dram_tensor(name, shape, dt, kind=kind).ap()

    emb = dram("emb", [V, E])
    idx_x = dram("idx_x", [NL, 1], I32)
    idx_e0 = dram("idx_e0", [NL, 1], I32)
    idx_e1 = dram("idx_e1", [NL, 1], I32)
    # weights pre-laid-out on host as lhsT tiles [p, kt, j]
    wgf = dram("wgf", [P, NE, 4 * G], BF16)
    wgr = dram("wgr", [P, NE, 4 * G], BF16)
    ugf = dram("ugf", [P, NH, 4 * G], BF16)
    ugr = dram("ugr", [P, NH, 4 * G], BF16)
    wf = dram("wf", [P, NE, 3 * E], BF16)   # i, g, o gates only (zero state)
    bgf = dram("bgf", [P, 8])
    bgr = dram("bgr", [P, 8])
    bf = dram("bf", [P, 12])
    out_a = dram("out_a", [P, 24], kind="ExternalOutput")
    out_d = dram("out_d", [P, 8], kind="ExternalOutput")

    with tile.TileContext(nc) as tc:
        _emit(tc, locals())
    nc.compile()
    return nc


def _emit(tc, T):
    nc = tc.nc
    rg = [list(range(NCORES))]
    from contextlib import ExitStack
    ctx = ExitStack()
    with ctx:
        glob = ctx.enter_context(tc.tile_pool(name="glob", bufs=1))
        dramp = ctx.enter_context(tc.tile_pool(name="dramp", bufs=1, space="DRAM"))

        ident = glob.tile([P, P], F32)
        make_identity(nc, ident)
        identb = glob.tile([P, P], BF16)
        nc.vector.tensor_copy(out=identb[:], in_=ident[:])

        # fp8 exchange buffers, partition-major 2KB rows
        ag1_src = dramp.tile([P, NE * NL], FP8)
        ag1_dst = dramp.tile([NCORES * P, NE * NL], FP8, addr_space="Shared")
        ag2a_src = dramp.tile([P, 4 * E], FP8)
        ag2a_dst = dramp.tile([NCORES * P, 4 * E], FP8, addr_space="Shared")
        ag2b_src = dramp.tile([P, 4 * E], FP8)
        ag2b_dst = dramp.tile([NCORES * P, 4 * E], FP8, addr_space="Shared")

        # long-lived activations
        g0T = glob.tile([P, NE, NL], BF16)
        g1T = glob.tile([P, NE, NL], BF16)
        dg8 = glob.tile([P, NE, NL], FP8)
        A0T = glob.tile([P, NMB, NL], FP8)
        out_act = glob.tile([P, 24], F32)
        out_dve = glob.tile([P, 8], F32)

        with tc.tile_pool(name="wpool", bufs=1) as wp, \
             tc.tile_pool(name="acts", bufs=1) as ap_, \
             tc.tile_pool(name="gates", bufs=1) as gp, \
             tc.tile_pool(name="tmp", bufs=1) as tp, \
             tc.tile_pool(name="idx", bufs=1) as ip, \
             tc.tile_pool(name="raw", bufs=1) as rp, \
             tc.tile_pool(name="fhk", bufs=1) as fkp, \
             tc.tile_pool(name="tps", bufs=1) as tsp:
            lstm_psum = tc.tile_pool(name="pg", bufs=1, space="PSUM")
            pgp = lstm_psum.__enter__()
            ptp_cm = tc.tile_pool(name="pt", bufs=1, space="PSUM")
            ptp = ptp_cm.__enter__()
            pools = {"pg": pgp, "gates": gp, "tmp": tp, "idx": ip,
                     "raw": rp, "pt": ptp}

            # ---- phase F: x gather, f-LSTM, AG1 as early as possible ----
            xT = ap_.tile([P, NE, NL], BF16)
            _gather_T(nc, pools, T["emb"], T["idx_x"], ident, xT)
            e0T = ap_.tile([P, NE, NL], BF16)
            e1T = ap_.tile([P, NE, NL], BF16)
            _gather_T(nc, pools, T["emb"], T["idx_e0"], ident, e0T)
            _gather_T(nc, pools, T["emb"], T["idx_e1"], ident, e1T)

            wf_sb = wp.tile([P, NE, 3 * E], BF16)
            for q in range(3):   # chunked so the i-gate matmuls start early
                qs = slice(q * E, (q + 1) * E)
                nc.sync.dma_start(out=wf_sb[:, :, qs], in_=T["wf"][:, :, qs])
            bf_sb = wp.tile([P, 12], F32)
            nc.sync.dma_start(out=bf_sb[:], in_=T["bf"][:])
            w_sb = {}
            for nm, kt in (("wgf", NE), ("wgr", NE), ("ugf", NH), ("ugr", NH)):
                w_sb[nm] = wp.tile([P, kt, 4 * G], BF16, name=nm + "_sb")
                for q in (0, 2, 3, 1):   # forget gate last (needed by stage 2)
                    qs = slice(q * G, (q + 1) * G)
                    nc.sync.dma_start(out=w_sb[nm][:, :, qs],
                                      in_=T[nm][:, :, qs])
            for nm in ("bgf", "bgr"):
                w_sb[nm] = wp.tile([P, 8], F32, name=nm + "_sb")
                nc.sync.dma_start(out=w_sb[nm][:], in_=T[nm][:])

            fhT = ap_.tile([P, NE, NL], BF16)
            _lstm_cell(nc, pools, E, xT, wf_sb, None, None, None, bf_sb, fhT,
                       None, gates=(0, 2, 3), packed=(0, 2, 3))
            fh8 = ap_.tile([P, NE, NL], FP8, name="fh8")
            for et in range(NE):
                nc.vector.tensor_add(fhT[:, et, :], fhT[:, et, :], xT[:, et, :])
                nc.vector.tensor_copy(out=fh8[:, et, :], in_=fhT[:, et, :])
            nc.sync.dma_start(
                out=ag1_src[:], in_=fh8[:].rearrange("p et n -> p (et n)"))
            nc.gpsimd.collective_compute(
                "AllGather", ALU.bypass, replica_groups=rg,
                ins=[ag1_src[:].opt()], outs=[ag1_dst[:].opt()])

            # ---- g-LSTM: stage 1 (zero state), then stage 2 ----
            cfT = ap_.tile([P, NH, NL], F32, name="cfT")
            crT = ap_.tile([P, NH, NL], F32, name="crT")
            c2T = ap_.tile([P, NH, NL], F32, name="c2T")
            c3T = ap_.tile([P, NH, NL], F32, name="c3T")
            hf0 = g0T[:, 0:NH, :]
            hf1 = g1T[:, 0:NH, :]
            hr1 = g1T[:, NH:NE, :]
            hr0 = g0T[:, NH:NE, :]
            _lstm_cell(nc, pools, G, e0T, w_sb["wgf"], None, None, None,
                       w_sb["bgf"], hf0, cfT, gates=(0, 2, 3))
            _lstm_cell(nc, pools, G, e1T, w_sb["wgr"], None, None, None,
                       w_sb["bgr"], hr1, crT, gates=(0, 2, 3))
            # forget-gate weight quarters land last; stage 2 needs them
            _lstm_cell(nc, pools, G, e1T, w_sb["wgf"], w_sb["ugf"], hf0, cfT,
                       w_sb["bgf"], hf1, c2T)
            # dg e-subtiles 0,1 (hf0-hf1) ready before rev0 completes
            for et in range(NH):
                nc.vector.tensor_sub(dg8[:, et, :], g0T[:, et, :],
                                     g1T[:, et, :])
            _lstm_cell(nc, pools, G, e0T, w_sb["wgr"], w_sb["ugr"], hr1, crT,
                       w_sb["bgr"], hr0, c3T)
            for et in range(NH, NE):
                nc.vector.tensor_sub(dg8[:, et, :], g0T[:, et, :],
                                     g1T[:, et, :])
            # sg / t1 reductions now: the Act engine is idle while AG1 runs
            for b, gT in ((0, g0T), (1, g1T)):
                for et in range(NE):
                    scr = tp.tile([P, NL], F32, tag="scr", bufs=2, name="scr")
                    c0 = 12 + 4 * b + et
                    nc.scalar.activation(
                        out=scr[:], in_=gT[:, et, :],
                        func=AF.Square, accum_out=out_act[:, c0:c0 + 1])
            for et in range(NE):
                scr = tp.tile([P, NL], F32, tag="scr", bufs=2, name="scr")
                nc.scalar.activation(
                    out=scr[:], in_=g1T[:, et, :],
                    func=AF.Copy, accum_out=out_act[:, 20 + et:20 + et + 1])
            ptp_cm.__exit__(None, None, None)
            lstm_psum.__exit__(None, None, None)
            pd_cm = tc.tile_pool(name="pd", bufs=1, space="PSUM")
            pdp = pd_cm.__enter__()

            # ---- transpose g0/g1 to row-major fp8, fire AG2a / AG2b ----
            with tc.high_priority():
                for srcT, s_sb, a_src, a_dst in (
                        (g0T, "s0", ag2a_src, ag2a_dst),
                        (g1T, "s1", ag2b_src, ag2b_dst)):
                    s_t = tsp.tile([P, NE, E], FP8, tag=s_sb, bufs=1, name=s_sb)
                    for nt in range(NL // P):
                        ptile = pdp.tile([P, E], BF16, tag="ptg", bufs=3,
                                         name="ptg")
                        for et in range(NE):
                            nc.tensor.transpose(
                                out=ptile[:, et * P:(et + 1) * P],
                                in_=srcT[:, et, nt * P:(nt + 1) * P],
                                identity=identb[:])
                        nc.vector.tensor_copy(out=s_t[:, nt, :], in_=ptile[:])
                    nc.sync.dma_start(
                        out=a_src[:], in_=s_t[:].rearrange("p s e -> p (s e)"))
                    nc.gpsimd.collective_compute(
                        "AllGather", ALU.bypass, replica_groups=rg,
                        ins=[a_src[:].opt()], outs=[a_dst[:].opt()])

            # ---- D1: A0 = sigmoid((g0-g1) @ fh_all.T), fp8 DoubleRow ----
            # fhk loads ride the gpsimd queue: they wait on AG1-done, and on
            # the SP queue that wait would block the ag2 source writes.
            for k in range(NCORES):
                fhk = fkp.tile([P, NE, NL], FP8, tag="fhk", bufs=3, name="fhk")
                nc.gpsimd.dma_start(
                    out=fhk[:],
                    in_=ag1_dst[k * P:(k + 1) * P, :].rearrange(
                        "p (et n) -> p et n", et=NE))
                for c in range(0, NL // P, 2):
                    mb = k * (NL // P) + c
                    pd = pdp.tile([P, 2, NL], F32, tag="pd", bufs=2, name="pd")
                    for h in range(2):
                        cs = slice((c + h) * P, (c + h + 1) * P)
                        nc.tensor.matmul(
                            pd[:, h, :], fhk[:, 0:2, cs], dg8[:, 0:2, :],
                            start=True, stop=False, perf_mode=DR)
                        nc.tensor.matmul(
                            pd[:, h, :], fhk[:, 2:4, cs], dg8[:, 2:4, :],
                            start=False, stop=True, perf_mode=DR)
                    nc.scalar.activation(
                        out=A0T[:, mb:mb + 2, :], in_=pd[:], func=AF.Sigmoid)
            pd_cm.__exit__(None, None, None)

        # ---- D2: r0 = A0@g0, q = A0@g1 (fp8 DoubleRow), reductions ----
        # et-outer with all 8 g-tiles resident: each et's PSUM accumulator
        # completes early so its reductions overlap the next et's matmuls.
        with tc.tile_pool(name="gb", bufs=1) as gbp, \
             tc.tile_pool(name="fin", bufs=1) as fin, \
             tc.tile_pool(name="pr", bufs=1, space="PSUM") as prp:
            for phase, (a_dst, gT, dcol0) in enumerate(
                    ((ag2a_dst, g0T, 0), (ag2b_dst, g1T, 4))):
                gpk = [gbp.tile([P, NE, E], FP8, tag=f"gpk{k}", bufs=2,
                                name=f"gpk{k}") for k in range(NCORES)]
                for k in range(NCORES):
                    nc.sync.dma_start(
                        out=gpk[k][:],
                        in_=a_dst[k * P:(k + 1) * P, :].rearrange(
                            "p (s e) -> p s e", s=NE))
                for et in range(NE):
                    rp_ = prp.tile([P, NL], F32, tag=f"r{et % 2}",
                                   bufs=2, name=f"r{et}")
                    es = slice(et * P, (et + 1) * P)
                    for k in range(NCORES):
                        for cp in range(2):
                            mp = k * 4 + 2 * cp
                            nc.tensor.matmul(
                                rp_[:], gpk[k][:, 2 * cp:2 * cp + 2, es],
                                A0T[:, mp:mp + 2, :],
                                start=(k == 0 and cp == 0),
                                stop=(k == NCORES - 1 and cp == 1),
                                perf_mode=DR)
                    scr = fin.tile([P, NL], F32, tag="scr", bufs=2, name="scr")
                    nc.scalar.activation(
                        out=scr[:], in_=rp_[:], func=AF.Square,
                        accum_out=out_act[:, 8 * phase + et:8 * phase + et + 1])
                    scr2 = fin.tile([P, NL], F32, tag="scr2", bufs=2,
                                    name="scr2")
                    nc.vector.scalar_tensor_tensor(
                        out=scr2[:], in0=rp_[:], scalar=1.0,
                        in1=gT[:, et, :],
                        op0=ALU.mult, op1=ALU.mult,
                        accum_out=out_dve[:, dcol0 + et:dcol0 + et + 1])
                    if phase == 1:
                        scr3 = fin.tile([P, NL], F32, tag="scr3", bufs=2,
                                        name="scr3")
                        nc.scalar.activation(
                            out=scr3[:], in_=rp_[:], func=AF.Copy,
                            accum_out=out_act[:, 4 + et:4 + et + 1])

            nc.sync.dma_start(out=T["out_a"][:], in_=out_act[:])
            nc.sync.dma_start(out=T["out_d"][:], in_=out_dve[:])


_PROGRAM = None


def _get_program():
    global _PROGRAM
    if _PROGRAM is None:
        _PROGRAM = build_program()
    return _PROGRAM


def _prep_w(w, gates=(0, 1, 2, 3)):
    """(4H, E_in) torch-layout weight -> bf16 lhsT tiles [p, kt, len(gates)*H]."""
    import ml_dtypes
    w = np.asarray(w, np.float32)
    h4 = w.shape[0]
    h = h4 // 4
    wt = np.concatenate([w[g * h:(g + 1) * h] for g in gates], 0).T
    e_in, cols = wt.shape
    return np.ascontiguousarray(
        wt.reshape(e_in // P, P, cols).transpose(1, 0, 2)
        .astype(ml_dtypes.bfloat16))


def _prep_b(b1, b2, gates=(0, 1, 2, 3)):
    s = np.asarray(b1, np.float32) + np.asarray(b2, np.float32)
    h = s.shape[0] // 4
    s = np.concatenate([s[g * h:(g + 1) * h] for g in gates], 0)
    return np.ascontiguousarray(s.reshape(-1, P).T)


def run_device(inputs, trace=False):
    """Shard inputs, run the 8-core SPMD program, return bass results."""
    nc = _get_program()
    emb = np.ascontiguousarray(np.asarray(inputs["embedding"], np.float32))
    iq = np.asarray(inputs["input"]).astype(np.int32).reshape(N, 1)
    ie = np.asarray(inputs["set_inputs"]).astype(np.int32)
    shared = {
        "emb": emb,
        "wgf": _prep_w(inputs["wih_gf"]), "wgr": _prep_w(inputs["wih_gr"]),
        "ugf": _prep_w(inputs["whh_gf"]), "ugr": _prep_w(inputs["whh_gr"]),
        "wf": _prep_w(inputs["wih_f"], gates=(0, 2, 3)),
        "bgf": _prep_b(inputs["bih_gf"], inputs["bhh_gf"]),
        "bgr": _prep_b(inputs["bih_gr"], inputs["bhh_gr"]),
        "bf": _prep_b(inputs["bih_f"], inputs["bhh_f"], gates=(0, 2, 3)),
    }
    in_maps = []
    for k in range(NCORES):
        sl = slice(k * NL, (k + 1) * NL)
        m = dict(shared)
        m["idx_x"] = np.ascontiguousarray(iq[sl])
        m["idx_e0"] = np.ascontiguousarray(ie[0, sl].reshape(NL, 1))
        m["idx_e1"] = np.ascontiguousarray(ie[1, sl].reshape(NL, 1))
        in_maps.append(m)
    res = bass_utils.run_bass_kernel_spmd(
        nc, in_maps, core_ids=list(range(NCORES)), trace=trace)
    return res


def kernel(**inputs):
    res = run_device(inputs)
    return host_tail(res, inputs)


def host_tail(res, inputs):
    # device partial layout:
    #   out_a cols: sr0(0:4) sqq(4:8)... see below; out_d cols: dot0, dot1q
    # column c of kind at base: value for e = et*128 + p
    sr0 = np.zeros(E, np.float64)
    sqq = np.zeros(E, np.float64)
    sumq = np.zeros(E, np.float64)
    sg0 = np.zeros(E, np.float64)
    sg1 = np.zeros(E, np.float64)
    t1 = np.zeros(E, np.float64)
    dot0 = np.zeros(E, np.float64)
    dot1q = np.zeros(E, np.float64)
    for r in res.results:
        a = np.asarray(r["out_a"], np.float64)   # (P, 24)
        d = np.asarray(r["out_d"], np.float64)   # (P, 8)
        for et in range(NE):
            sl = slice(et * P, (et + 1) * P)
            sr0[sl] += a[:, 0 + et]
            sumq[sl] += a[:, 4 + et]
            sqq[sl] += a[:, 8 + et]
            sg0[sl] += a[:, 12 + et]
            sg1[sl] += a[:, 16 + et]
            t1[sl] += a[:, 20 + et]
            dot0[sl] += d[:, et]
            dot1q[sl] += d[:, 4 + et]
    s1 = t1
    dot1 = s1 * t1 - dot1q
    sr1 = N * s1 ** 2 - 2.0 * s1 * sumq + sqq
    dot = np.stack([dot0, dot1])
    sr = np.stack([sr0, sr1])
    sg = np.stack([sg0, sg1])
    nr = np.maximum(np.sqrt(sr), EPS)
    ng = np.maximum(np.sqrt(sg), EPS)
    cos = dot / (nr * ng)
    kern = cos / np.exp(cos).sum()
    w_out = np.asarray(inputs["w_out"], np.float64)
    b_out = np.asarray(inputs["b_out"], np.float64)
    k2 = kern @ w_out.T + b_out
    s = k2.sum(axis=1)
    labels = np.asarray(inputs["set_labels"], np.float64)
    o = s[0] * labels[0] + s[1] * labels[1]
    o = np.exp(o - o.max())
    o /= o.sum()
    return o.astype(np.float32)
